# revision 1
# baseline (speedup 1.0000x reference)
"""CapLayer (grouped 1x1 conv + capsule dynamic routing), data-parallel over batch
across 8 NeuronCores.

Strategy (per sharding hint): batch 256 -> 32 per core; conv weight (5120x8)
replicated; routing is batch-local so no cross-device communication.
"""

import numpy as np

NUM_SHARED = 32
IN_DIM = 8
NUM_OUT_CAPS = 10
OUT_DIM = 16
ROUTE_NUM = 3
N_CORES = 8

_BS, _C, _H, _W = 256, 256, 6, 6


def _caplayer_block(x, W, bias):
    """x: (bs_l, C, h, w) on one device. Returns v: (bs_l, J, D).

    Factorized routing: pred[b,(g,p),j,d] = sum_i Wt[g,j,d,i] xt[b,g,i,p] is
    never materialized; both routing contractions are reassociated through the
    9-wide (8 chans + folded bias) input instead. Exact reassociation of the
    reference einsums.
    """
    import jax.numpy as jnp

    bs = x.shape[0]
    G, J, D, din = NUM_SHARED, NUM_OUT_CAPS, OUT_DIM, IN_DIM
    hw = _H * _W
    xg = x.reshape(bs, G, din, hw)
    # augmented input: 9th "channel" of ones carries the bias
    xt = jnp.concatenate([xg, jnp.ones((bs, G, 1, hw), dtype=x.dtype)], axis=2)
    # augmented weight, laid out (G, J, din+1, D) so the routing einsums
    # contract the trailing dims without compiler-inserted transposes
    Wt = jnp.concatenate(
        [W.reshape(G, J, D, din), bias.reshape(G, J, D, 1)], axis=3
    ).transpose(0, 1, 3, 2)

    L = None
    v = None
    for t in range(ROUTE_NUM):
        if t == 0:
            # L = 0 -> c = 1/J uniform: z = (1/J) * sum_p xt, same for all j
            z = jnp.broadcast_to(
                (1.0 / J) * jnp.sum(xt, axis=3)[:, None, :, :], (bs, J, G, din + 1)
            )
        else:
            # logits are bounded (|L| ~ 10), so softmax without max-subtraction
            e = jnp.exp(L)
            c = e / jnp.sum(e, axis=1, keepdims=True)
            # z[b,j,g,i] = sum_p c[b,j,g,p] xt[b,g,i,p]
            z = jnp.einsum('bjgp,bgip->bjgi', c, xt)
        # s[b,j,d] = sum_{g,i} Wt[g,j,d,i] z[b,j,g,i]
        s = jnp.einsum('bjgi,gjid->bjd', z, Wt)
        norm2 = jnp.sum(s * s, axis=2)
        coeff = norm2 / (1.0 + norm2) / jnp.sqrt(norm2)
        v = s * coeff[:, :, None]
        # delta[b,j,g,p] = sum_i (sum_d v[b,j,d] Wt[g,j,d,i]) xt[b,g,i,p]
        if t < ROUTE_NUM - 1:
            vW = jnp.einsum('bjd,gjid->bjgi', v, Wt)
            delta = jnp.einsum('bjgi,bgip->bjgp', vW, xt)
            L = delta if L is None else L + delta
    return v


def _run_sharded(x, W, bias):
    import jax
    import jax.numpy as jnp
    from jax.sharding import Mesh, PartitionSpec as P
    from jax.experimental.shard_map import shard_map

    devs = jax.devices()[:N_CORES]
    mesh = Mesh(np.array(devs), ('x',))
    fn = shard_map(
        _caplayer_block,
        mesh=mesh,
        in_specs=(P('x'), P(), P()),
        out_specs=P('x'),
    )
    fn = jax.jit(fn)
    out = fn(jnp.asarray(x), jnp.asarray(W), jnp.asarray(bias))
    return np.asarray(out)


def _run_cpu(x, W, bias):
    G, J, D, din = NUM_SHARED, NUM_OUT_CAPS, OUT_DIM, IN_DIM
    bs = x.shape[0]
    hw = _H * _W
    xg = x.reshape(bs, G, din, hw)
    Wg = W.reshape(G, J * D, din)
    raw = np.einsum('bgip,goi->bgop', xg, Wg, optimize=True) + bias.reshape(G, J * D, 1)
    pred = raw.reshape(bs, G, J, D, hw).transpose(0, 1, 4, 2, 3).reshape(bs, G * hw, J, D)
    b = np.zeros((bs, J, G * hw), dtype=pred.dtype)
    v = None
    for _ in range(ROUTE_NUM):
        m = b.max(axis=1, keepdims=True)
        c = np.exp(b - m)
        c /= c.sum(axis=1, keepdims=True)
        s = np.einsum('bji,bijd->bjd', c, pred, optimize=True)
        norm2 = (s * s).sum(axis=2)
        coeff = norm2 / (1.0 + norm2) / np.sqrt(norm2)
        v = s * coeff[:, :, None]
        b = b + np.einsum('bjd,bijd->bji', v, pred, optimize=True)
    return v


def kernel(x, W, bias):
    x = np.ascontiguousarray(x, dtype=np.float32)
    W = np.ascontiguousarray(W, dtype=np.float32)
    bias = np.ascontiguousarray(bias, dtype=np.float32)
    try:
        return _run_sharded(x, W, bias).astype(np.float32)
    except Exception:
        return _run_cpu(x, W, bias).astype(np.float32)

